# revision 7
# baseline (speedup 1.0000x reference)
"""BertTinyFlatten on 8 Trainium2 NeuronCores — data-parallel over batch.

Per core (one batch element):
  emb   = gather(word_emb, x) + (pos_emb + tok_emb[0])      [indirect DMA w/ CCE add]
  x0    = layernorm(emb)                                     [token-major, DVE/ACT]
  x0t   = x0.T (PE transpose, LN affine fused into PSUM->SBUF copy)
  y1    = x0 @ init_d.T          (token-major out)           [f32r matmuls]
  y1sq  = (mix(y1, init_M) + b1)^2    -> feature-major       [ACT Square fused]
  y2    = y1sq-chain @ inter0_d.T     -> token-major
  y2sq  = (mix(y2, inter0_M) + b2)^2  -> feature-major
  yt    = final_d-chain @ y2sq + b3   -> feature-major; host transposes back

All matmuls run as float32r (FP32 truncated to ~FP22 inside the PE) at
bf16 rate: 1 cycle/row for N=512.
"""
import os
import sys

import numpy as np
import ml_dtypes

for _p in ("/opt/trn_rl_repo", "/opt/pypackages"):
    if _p not in sys.path and os.path.isdir(_p):
        sys.path.append(_p)

from contextlib import ExitStack

import concourse.bass as bass
import concourse.tile as tile
from concourse import bacc, masks, mybir
from concourse.bass import IndirectOffsetOnAxis
from concourse.bass_utils import run_bass_kernel_spmd

f32 = mybir.dt.float32
f32r = mybir.dt.float32r
bf16 = mybir.dt.bfloat16
i32 = mybir.dt.int32
AF = mybir.ActivationFunctionType
ALU = mybir.AluOpType
AX = mybir.AxisListType

B, S, HID, NH, INTER, VOCAB = 8, 1024, 512, 8, 2048, 30522
DH = INTER // NH            # 256 features per head
EPS = 1e-12
N_CORES = 8

KH = HID // 128             # 4   k-tiles for dense1
KI = INTER // 128           # 16  k-tiles for dense2/3
SC = S // 128               # 8   token chunks
NC1 = INTER // 512          # 4   n-chunks (512) for dense1/2
HT = HID // 128             # 4   hid tiles for dense3

STAGES = ("A", "B", "C", "D", "E", "full")


def _build_program(stage="full"):
    upto = STAGES.index(stage)
    nc = bacc.Bacc("TRN2", target_bir_lowering=False, debug=False,
                   num_devices=N_CORES)

    xw = nc.dram_tensor("xw", [128, SC], i32, kind="ExternalInput").ap()
    word_emb = nc.dram_tensor("word_emb", [VOCAB, HID], f32, kind="ExternalInput").ap()
    posplus = nc.dram_tensor("posplus", [S, HID], f32, kind="ExternalInput").ap()
    lnw = nc.dram_tensor("lnw", [128, HT], f32, kind="ExternalInput").ap()
    lnb = nc.dram_tensor("lnb", [128, HT], f32, kind="ExternalInput").ap()
    w1t = nc.dram_tensor("w1t", [HID, INTER], f32, kind="ExternalInput").ap()
    b1c = nc.dram_tensor("b1c", [128, KI], f32, kind="ExternalInput").ap()
    m1 = nc.dram_tensor("m1", [NH, S, S], bf16, kind="ExternalInput").ap()
    w2t = nc.dram_tensor("w2t", [INTER, INTER], f32, kind="ExternalInput").ap()
    b2c = nc.dram_tensor("b2c", [128, KI], f32, kind="ExternalInput").ap()
    m2 = nc.dram_tensor("m2", [NH, S, S], bf16, kind="ExternalInput").ap()
    w3t = nc.dram_tensor("w3t", [INTER, HID], f32, kind="ExternalInput").ap()
    b3c = nc.dram_tensor("b3c", [128, HT], f32, kind="ExternalInput").ap()
    yt_out = nc.dram_tensor("yt", [HID, S], f32, kind="ExternalOutput").ap()

    with tile.TileContext(nc) as tc, ExitStack() as ctx:
        pool = ctx.enter_context(tc.tile_pool(name="sbuf", bufs=1))
        psum = ctx.enter_context(tc.tile_pool(name="psum", bufs=1, space="PSUM"))

        def dump(tiles):
            # debug: write four [128, >=S] tiles to yt_out
            for i, t in enumerate(tiles[:4]):
                nc.sync.dma_start(yt_out[i * 128:(i + 1) * 128, :],
                                  t[:, 0:S].bitcast(f32))

        # ---- token ids first: the gathers depend only on this ----------
        t_idx = pool.tile([128, SC], i32)
        nc.sync.dma_start(t_idx[:], xw[:])

        # ---- decoupled embedding gathers (gpsimd queue, start early) ---
        gw = []
        for c in range(SC):
            g = pool.tile([128, HID], f32, tag="gword", bufs=SC, name=f"gw{c}")
            nc.gpsimd.indirect_dma_start(
                out=g[:], out_offset=None,
                in_=word_emb[:128, :],
                in_offset=IndirectOffsetOnAxis(ap=t_idx[:, c:c + 1], axis=0),
                bounds_check=VOCAB - 1, oob_is_err=False,
            )
            gw.append(g)

        # ---- posplus chunks (sync queue, parallel with gathers) --------
        emb = []
        for c in range(SC):
            e = pool.tile([128, HID], f32, tag="emb", bufs=SC, name=f"emb{c}")
            nc.sync.dma_start(e[:], posplus[c * 128:(c + 1) * 128, :])
            emb.append(e)

        # ---- constants -------------------------------------------------
        ident = pool.tile([128, 128], f32)
        masks.make_identity(nc, ident[:])
        zerocol = pool.tile([128, 1], f32)
        nc.vector.memset(zerocol[:], 0.0)
        epscol = pool.tile([128, 1], f32)
        nc.vector.memset(epscol[:], EPS)
        t_lnw = pool.tile([128, HT], f32)
        nc.sync.dma_start(t_lnw[:], lnw[:])
        t_lnb = pool.tile([128, HT], f32)
        nc.sync.dma_start(t_lnb[:], lnb[:])
        t_b1 = pool.tile([128, KI], f32)
        nc.sync.dma_start(t_b1[:], b1c[:])
        t_b2 = pool.tile([128, KI], f32)
        nc.sync.dma_start(t_b2[:], b2c[:])
        t_b3 = pool.tile([128, HT], f32)
        nc.sync.dma_start(t_b3[:], b3c[:])
        # ---- stage A: add gathered words + layernorm -------------------
        for c in range(SC):
            e = emb[c]
            nc.vector.tensor_add(e[:], e[:], gw[c][:])
            msum = pool.tile([128, 1], f32, tag="msum", bufs=2, name=f"msum{c}")
            nc.vector.reduce_sum(msum[:], e[:], axis=AX.X)
            mean = pool.tile([128, 1], f32, tag="mean", bufs=2, name=f"mean{c}")
            nc.scalar.mul(mean[:], msum[:], 1.0 / HID)
            sqd = pool.tile([128, HID], f32, tag="sqd", bufs=1, name=f"sqd{c}")
            ssq = pool.tile([128, 1], f32, tag="ssq", bufs=2, name=f"ssq{c}")
            nc.scalar.activation(sqd[:], e[:], AF.Square, bias=zerocol[:],
                                 accum_out=ssq[:])
            msq = pool.tile([128, 1], f32, tag="msq", bufs=2, name=f"msq{c}")
            nc.vector.tensor_mul(msq[:], mean[:], mean[:])
            var = pool.tile([128, 1], f32, tag="var", bufs=2, name=f"var{c}")
            nc.vector.tensor_scalar(var[:], ssq[:], 1.0 / HID, None, op0=ALU.mult)
            nc.vector.tensor_sub(var[:], var[:], msq[:])
            std = pool.tile([128, 1], f32, tag="std", bufs=2, name=f"std{c}")
            nc.scalar.activation(std[:], var[:], AF.Sqrt, bias=epscol[:])
            rstd = pool.tile([128, 1], f32, tag="rstd", bufs=2, name=f"rstd{c}")
            nc.vector.reciprocal(rstd[:], std[:])
            nc.vector.tensor_scalar(e[:], e[:], mean[:], rstd[:],
                                    op0=ALU.subtract, op1=ALU.mult)

        # feature-major activations live in the 16-slot "featmaj" ring:
        # x0t (4 tiles) -> y1sq (16) -> y2sq (16), WAR-serialized by Tile.
        x0t = []
        for ht in range(HT):
            x0t.append(pool.tile([128, S], f32r, tag="featmaj", bufs=16,
                                 name=f"x0t{ht}"))
        for c in range(SC):
            for ht in range(HT):
                pt = psum.tile([128, 512], f32, tag="mm", bufs=8, name=f"ptr{ht}_{c}")
                nc.tensor.transpose(pt[:, 0:128], emb[c][:, ht * 128:(ht + 1) * 128],
                                    ident[:])
                nc.vector.tensor_scalar(
                    x0t[ht][:, c * 128:(c + 1) * 128], pt[:, 0:128],
                    t_lnw[:, ht:ht + 1], t_lnb[:, ht:ht + 1],
                    op0=ALU.mult, op1=ALU.add)

        def mix(yin, m_ap, bias_tile, out_name):
            # per-head seq mix + bias + square; token-major in, feature-major out
            ysq = []
            for h in range(NH):
                groups = [[None] * 2 for _ in range(2)]
                for tc_i in range(2):
                    for dp in range(2):
                        groups[tc_i][dp] = psum.tile(
                            [128, 512], f32, tag="mm", bufs=8,
                            name=f"{out_name}p{h}_{tc_i}_{dp}")
                for s in range(SC):
                    mt = pool.tile([128, S], bf16, tag="mring", bufs=8,
                                   name=f"{out_name}m{h}_{s}")
                    nc.sync.dma_start(mt[:], m_ap[h, s * 128:(s + 1) * 128, :])
                    for dp in range(2):
                        lhsT = yin[s][:, h * DH + dp * 128: h * DH + (dp + 1) * 128]
                        for tc_i in range(2):
                            nc.tensor.matmul(groups[tc_i][dp][:], lhsT,
                                             mt[:, tc_i * 512:(tc_i + 1) * 512],
                                             start=(s == 0), stop=(s == SC - 1))
                for dp in range(2):
                    i = h * 2 + dp
                    yo = pool.tile([128, S], f32r, tag="featmaj", bufs=16,
                                   name=f"{out_name}{i}")
                    for tc_i in range(2):
                        nc.scalar.activation(yo[:, tc_i * 512:(tc_i + 1) * 512],
                                             groups[tc_i][dp][:], AF.Square,
                                             bias=bias_tile[:, i:i + 1])
                    ysq.append(yo)
            return ysq

        def dense(xin, w_ap, nk, nm):
            # token-major out: y[s, n] = x @ w  (xin: feature-major tiles)
            yt = []
            for s in range(SC):
                yt.append(pool.tile([128, INTER], bf16, tag="tokmaj", bufs=SC,
                                    name=f"{nm}{s}"))
            for n in range(NC1):
                wts = []
                for k in range(nk):
                    wt = pool.tile([128, 512], f32r, tag="wring", bufs=12,
                                   name=f"{nm}w{n}_{k}")
                    nc.sync.dma_start(wt[:], w_ap[k * 128:(k + 1) * 128,
                                                  n * 512:(n + 1) * 512]
                                      .bitcast(f32r))
                    wts.append(wt)
                for s in range(SC):
                    ps = psum.tile([128, 512], f32, tag="mm", bufs=8,
                                   name=f"{nm}p{n}_{s}")
                    for k in range(nk):
                        nc.tensor.matmul(ps[:], xin[k][:, s * 128:(s + 1) * 128],
                                         wts[k][:], start=(k == 0),
                                         stop=(k == nk - 1))
                    nc.scalar.copy(yt[s][:, n * 512:(n + 1) * 512], ps[:])
            return yt

        if upto == 0:                       # stage A only
            dump(x0t)
        if upto >= 1:
            y1 = dense(x0t, w1t, KH, "y1_")
            if upto == 1:
                dump(y1)
        if upto >= 2:
            y1sq = mix(y1, m1, t_b1, "y1sq")
            if upto == 2:
                dump(y1sq)
        if upto >= 3:
            y2 = dense(y1sq, w2t, KI, "y2_")
            if upto == 3:
                dump(y2)
        if upto >= 4:
            y2sq = mix(y2, m2, t_b2, "y2sq")
            if upto == 4:
                dump(y2sq)
        if upto >= 5:                       # dense3 + bias + store
            for ht in range(HT):
                yo = pool.tile([128, S], f32, tag="out", bufs=2, name=f"yt{ht}")
                pss = [psum.tile([128, 512], f32, tag="mm", bufs=8,
                                 name=f"p3_{ht}_{sc}") for sc in range(2)]
                for k in range(KI):
                    wt = pool.tile([128, 128], f32r, tag="w3ring", bufs=8,
                                   name=f"w3_{ht}_{k}")
                    nc.sync.dma_start(wt[:], w3t[k * 128:(k + 1) * 128,
                                                 ht * 128:(ht + 1) * 128]
                                      .bitcast(f32r))
                    for sc in range(2):
                        nc.tensor.matmul(pss[sc][:], wt[:],
                                         y2sq[k][:, sc * 512:(sc + 1) * 512],
                                         start=(k == 0), stop=(k == KI - 1))
                for sc in range(2):
                    nc.scalar.activation(yo[:, sc * 512:(sc + 1) * 512], pss[sc][:],
                                         AF.Identity, bias=t_b3[:, ht:ht + 1])
                    nc.sync.dma_start(
                        yt_out[ht * 128:(ht + 1) * 128, sc * 512:(sc + 1) * 512],
                        yo[:, sc * 512:(sc + 1) * 512])

    nc.compile()
    return nc


_PROGRAMS = {}
LAST_RESULT = None


def _get_program(stage="full"):
    if stage not in _PROGRAMS:
        _PROGRAMS[stage] = _build_program(stage)
    return _PROGRAMS[stage]


def _prep_maps(x, word_emb, pos_emb, tok_emb, emb_ln_w, emb_ln_b,
               init_d, init_b, init_M, inter0_d, inter0_b, inter0_M,
               final_d, final_b):
    x = np.asarray(x)
    f = lambda a: np.ascontiguousarray(np.asarray(a), dtype=np.float32)
    shared = dict(
        word_emb=f(word_emb),
        posplus=f(pos_emb) + f(tok_emb)[0][None, :],
        lnw=np.ascontiguousarray(f(emb_ln_w).reshape(HT, 128).T),
        lnb=np.ascontiguousarray(f(emb_ln_b).reshape(HT, 128).T),
        w1t=np.ascontiguousarray(f(init_d).T),
        b1c=np.ascontiguousarray(f(init_b).reshape(KI, 128).T),
        m1=np.ascontiguousarray(np.asarray(init_M)).astype(ml_dtypes.bfloat16),
        w2t=np.ascontiguousarray(f(inter0_d).T),
        b2c=np.ascontiguousarray(f(inter0_b).reshape(KI, 128).T),
        m2=np.ascontiguousarray(np.asarray(inter0_M)).astype(ml_dtypes.bfloat16),
        w3t=np.ascontiguousarray(f(final_d).T),
        b3c=np.ascontiguousarray(f(final_b).reshape(HT, 128).T),
    )
    in_maps = []
    for b in range(B):
        xwb = np.ascontiguousarray(x[b].astype(np.int32).reshape(SC, 128).T)
        in_maps.append(dict(shared, xw=xwb))
    return in_maps


def kernel(**inputs):
    global LAST_RESULT
    stage = os.environ.get("KSTAGE", "full")
    ncores = int(os.environ.get("KCORES", str(N_CORES)))
    in_maps = _prep_maps(**inputs)[:ncores]
    nc = _get_program(stage)
    res = run_bass_kernel_spmd(nc, in_maps, list(range(ncores)))
    LAST_RESULT = res
    out = np.stack([res.results[b]["yt"].T for b in range(ncores)])
    if ncores < B:
        out = np.concatenate([out] + [out[:1]] * (B - ncores))
    return out


# revision 8
# speedup vs baseline: 1.0380x; 1.0380x over previous
"""BertTinyFlatten on 8 Trainium2 NeuronCores — data-parallel over batch.

Per core (one batch element):
  emb   = gather(word_emb, x) + (pos_emb + tok_emb[0])      [indirect DMA w/ CCE add]
  x0    = layernorm(emb)                                     [token-major, DVE/ACT]
  x0t   = x0.T (PE transpose, LN affine fused into PSUM->SBUF copy)
  y1    = x0 @ init_d.T          (token-major out)           [f32r matmuls]
  y1sq  = (mix(y1, init_M) + b1)^2    -> feature-major       [ACT Square fused]
  y2    = y1sq-chain @ inter0_d.T     -> token-major
  y2sq  = (mix(y2, inter0_M) + b2)^2  -> feature-major
  yt    = final_d-chain @ y2sq + b3   -> feature-major; host transposes back

All matmuls run as float32r (FP32 truncated to ~FP22 inside the PE) at
bf16 rate: 1 cycle/row for N=512.
"""
import os
import sys

import numpy as np
import ml_dtypes

for _p in ("/opt/trn_rl_repo", "/opt/pypackages"):
    if _p not in sys.path and os.path.isdir(_p):
        sys.path.append(_p)

from contextlib import ExitStack

import concourse.bass as bass
import concourse.tile as tile
from concourse import bacc, masks, mybir
from concourse.bass import IndirectOffsetOnAxis
from concourse.bass_utils import run_bass_kernel_spmd

f32 = mybir.dt.float32
f32r = mybir.dt.float32r
bf16 = mybir.dt.bfloat16
i32 = mybir.dt.int32
AF = mybir.ActivationFunctionType
ALU = mybir.AluOpType
AX = mybir.AxisListType

B, S, HID, NH, INTER, VOCAB = 8, 1024, 512, 8, 2048, 30522
DH = INTER // NH            # 256 features per head
EPS = 1e-12
N_CORES = 8

KH = HID // 128             # 4   k-tiles for dense1
KI = INTER // 128           # 16  k-tiles for dense2/3
SC = S // 128               # 8   token chunks
NC1 = INTER // 512          # 4   n-chunks (512) for dense1/2
HT = HID // 128             # 4   hid tiles for dense3

STAGES = ("A", "B", "C", "D", "E", "full")


def _build_program(stage="full"):
    upto = STAGES.index(stage)
    nc = bacc.Bacc("TRN2", target_bir_lowering=False, debug=False,
                   num_devices=N_CORES)

    xw = nc.dram_tensor("xw", [128, SC], i32, kind="ExternalInput").ap()
    word_emb = nc.dram_tensor("word_emb", [VOCAB, HID], f32, kind="ExternalInput").ap()
    posplus = nc.dram_tensor("posplus", [S, HID], f32, kind="ExternalInput").ap()
    lnw = nc.dram_tensor("lnw", [128, HT], f32, kind="ExternalInput").ap()
    lnb = nc.dram_tensor("lnb", [128, HT], f32, kind="ExternalInput").ap()
    w1t = nc.dram_tensor("w1t", [HID, INTER], f32, kind="ExternalInput").ap()
    b1c = nc.dram_tensor("b1c", [128, KI], f32, kind="ExternalInput").ap()
    m1 = nc.dram_tensor("m1", [NH, S, S], bf16, kind="ExternalInput").ap()
    w2t = nc.dram_tensor("w2t", [INTER, INTER], f32, kind="ExternalInput").ap()
    b2c = nc.dram_tensor("b2c", [128, KI], f32, kind="ExternalInput").ap()
    m2 = nc.dram_tensor("m2", [NH, S, S], bf16, kind="ExternalInput").ap()
    w3t = nc.dram_tensor("w3t", [INTER, HID], f32, kind="ExternalInput").ap()
    b3c = nc.dram_tensor("b3c", [128, HT], f32, kind="ExternalInput").ap()
    yt_out = nc.dram_tensor("yt", [HID, S], f32, kind="ExternalOutput").ap()

    with tile.TileContext(nc) as tc, ExitStack() as ctx:
        pool = ctx.enter_context(tc.tile_pool(name="sbuf", bufs=1))
        psum = ctx.enter_context(tc.tile_pool(name="psum", bufs=1, space="PSUM"))

        def dump(tiles):
            # debug: write four [128, >=S] tiles to yt_out
            for i, t in enumerate(tiles[:4]):
                nc.sync.dma_start(yt_out[i * 128:(i + 1) * 128, :],
                                  t[:, 0:S].bitcast(f32))

        # ---- token ids first: the gathers depend only on this ----------
        t_idx = pool.tile([128, SC], i32)
        nc.sync.dma_start(t_idx[:], xw[:])

        # ---- decoupled embedding gathers (gpsimd queue, start early) ---
        gw = []
        for c in range(SC):
            g = pool.tile([128, HID], f32, tag="gword", bufs=SC, name=f"gw{c}")
            nc.gpsimd.indirect_dma_start(
                out=g[:], out_offset=None,
                in_=word_emb[:128, :],
                in_offset=IndirectOffsetOnAxis(ap=t_idx[:, c:c + 1], axis=0),
                bounds_check=VOCAB - 1, oob_is_err=False,
            )
            gw.append(g)

        # ---- posplus chunks (sync queue, parallel with gathers) --------
        emb = []
        for c in range(SC):
            e = pool.tile([128, HID], f32, tag="emb", bufs=SC, name=f"emb{c}")
            nc.sync.dma_start(e[:], posplus[c * 128:(c + 1) * 128, :])
            emb.append(e)

        # ---- constants -------------------------------------------------
        ident = pool.tile([128, 128], f32)
        masks.make_identity(nc, ident[:])
        zerocol = pool.tile([128, 1], f32)
        nc.vector.memset(zerocol[:], 0.0)
        epscol = pool.tile([128, 1], f32)
        nc.vector.memset(epscol[:], EPS)
        t_lnw = pool.tile([128, HT], f32)
        nc.sync.dma_start(t_lnw[:], lnw[:])
        t_lnb = pool.tile([128, HT], f32)
        nc.sync.dma_start(t_lnb[:], lnb[:])
        t_b1 = pool.tile([128, KI], f32)
        nc.sync.dma_start(t_b1[:], b1c[:])
        t_b2 = pool.tile([128, KI], f32)
        nc.sync.dma_start(t_b2[:], b2c[:])
        t_b3 = pool.tile([128, HT], f32)
        nc.sync.dma_start(t_b3[:], b3c[:])
        # ---- stage A: add gathered words + layernorm -------------------
        for c in range(SC):
            e = emb[c]
            nc.vector.tensor_add(e[:], e[:], gw[c][:])
            msum = pool.tile([128, 1], f32, tag="msum", bufs=2, name=f"msum{c}")
            nc.vector.reduce_sum(msum[:], e[:], axis=AX.X)
            mean = pool.tile([128, 1], f32, tag="mean", bufs=2, name=f"mean{c}")
            nc.scalar.mul(mean[:], msum[:], 1.0 / HID)
            sqd = pool.tile([128, HID], f32, tag="sqd", bufs=1, name=f"sqd{c}")
            ssq = pool.tile([128, 1], f32, tag="ssq", bufs=2, name=f"ssq{c}")
            nc.scalar.activation(sqd[:], e[:], AF.Square, bias=zerocol[:],
                                 accum_out=ssq[:])
            msq = pool.tile([128, 1], f32, tag="msq", bufs=2, name=f"msq{c}")
            nc.vector.tensor_mul(msq[:], mean[:], mean[:])
            var = pool.tile([128, 1], f32, tag="var", bufs=2, name=f"var{c}")
            nc.vector.tensor_scalar(var[:], ssq[:], 1.0 / HID, None, op0=ALU.mult)
            nc.vector.tensor_sub(var[:], var[:], msq[:])
            std = pool.tile([128, 1], f32, tag="std", bufs=2, name=f"std{c}")
            nc.scalar.activation(std[:], var[:], AF.Sqrt, bias=epscol[:])
            rstd = pool.tile([128, 1], f32, tag="rstd", bufs=2, name=f"rstd{c}")
            nc.vector.reciprocal(rstd[:], std[:])
            nc.vector.tensor_scalar(e[:], e[:], mean[:], rstd[:],
                                    op0=ALU.subtract, op1=ALU.mult)

        # feature-major activations live in the 16-slot "featmaj" ring:
        # x0t (4 tiles) -> y1sq (16) -> y2sq (16), WAR-serialized by Tile.
        x0t = []
        for ht in range(HT):
            x0t.append(pool.tile([128, S], f32r, tag="featmaj", bufs=16,
                                 name=f"x0t{ht}"))

        def mix(yin, m_ap, bias_tile, out_name):
            # per-head seq mix + bias + square; token-major in, feature-major out
            ysq = []
            for h in range(NH):
                groups = [[None] * 2 for _ in range(2)]
                for tc_i in range(2):
                    for dp in range(2):
                        groups[tc_i][dp] = psum.tile(
                            [128, 512], f32, tag="mm", bufs=8,
                            name=f"{out_name}p{h}_{tc_i}_{dp}")
                for s in range(SC):
                    mt = pool.tile([128, S], bf16, tag="mring", bufs=8,
                                   name=f"{out_name}m{h}_{s}")
                    nc.sync.dma_start(mt[:], m_ap[h, s * 128:(s + 1) * 128, :])
                    for dp in range(2):
                        lhsT = yin[s][:, h * DH + dp * 128: h * DH + (dp + 1) * 128]
                        for tc_i in range(2):
                            nc.tensor.matmul(groups[tc_i][dp][:], lhsT,
                                             mt[:, tc_i * 512:(tc_i + 1) * 512],
                                             start=(s == 0), stop=(s == SC - 1))
                for dp in range(2):
                    i = h * 2 + dp
                    yo = pool.tile([128, S], f32r, tag="featmaj", bufs=16,
                                   name=f"{out_name}{i}")
                    for tc_i in range(2):
                        nc.scalar.activation(yo[:, tc_i * 512:(tc_i + 1) * 512],
                                             groups[tc_i][dp][:], AF.Square,
                                             bias=bias_tile[:, i:i + 1])
                    ysq.append(yo)
            return ysq

        def dense(xin, w_ap, nk, nm, transpose_src=None):
            # token-major out: y[s, n] = x @ w  (xin: feature-major tiles).
            # With transpose_src, loop s-major and emit the PE transposes that
            # produce xin[:, s] right before the s-group (stage A/B fusion).
            yt = []
            for s in range(SC):
                yt.append(pool.tile([128, INTER], bf16, tag="tokmaj", bufs=SC,
                                    name=f"{nm}{s}"))
            if transpose_src is None:
                for n in range(NC1):
                    wts = []
                    for k in range(nk):
                        wt = pool.tile([128, 512], f32r, tag="wring", bufs=16,
                                       name=f"{nm}w{n}_{k}")
                        nc.sync.dma_start(wt[:], w_ap[k * 128:(k + 1) * 128,
                                                      n * 512:(n + 1) * 512]
                                          .bitcast(f32r))
                        wts.append(wt)
                    for s in range(SC):
                        ps = psum.tile([128, 512], f32, tag="mm", bufs=8,
                                       name=f"{nm}p{n}_{s}")
                        for k in range(nk):
                            nc.tensor.matmul(ps[:], xin[k][:, s * 128:(s + 1) * 128],
                                             wts[k][:], start=(k == 0),
                                             stop=(k == nk - 1))
                        nc.scalar.copy(yt[s][:, n * 512:(n + 1) * 512], ps[:])
                return yt
            # fused: preload all nk*NC1 weight tiles, then s-major
            wts = {}
            for n in range(NC1):
                for k in range(nk):
                    wt = pool.tile([128, 512], f32r, tag="wring", bufs=16,
                                   name=f"{nm}w{n}_{k}")
                    nc.sync.dma_start(wt[:], w_ap[k * 128:(k + 1) * 128,
                                                  n * 512:(n + 1) * 512]
                                      .bitcast(f32r))
                    wts[(n, k)] = wt
            for s in range(SC):
                for ht in range(HT):
                    pt = psum.tile([128, 512], f32, tag="mm", bufs=8,
                                   name=f"ptr{ht}_{s}")
                    nc.tensor.transpose(pt[:, 0:128],
                                        transpose_src[s][:, ht * 128:(ht + 1) * 128],
                                        ident[:])
                    nc.vector.tensor_scalar(
                        xin[ht][:, s * 128:(s + 1) * 128], pt[:, 0:128],
                        t_lnw[:, ht:ht + 1], t_lnb[:, ht:ht + 1],
                        op0=ALU.mult, op1=ALU.add)
                for n in range(NC1):
                    ps = psum.tile([128, 512], f32, tag="mm", bufs=8,
                                   name=f"{nm}p{n}_{s}")
                    for k in range(nk):
                        nc.tensor.matmul(ps[:], xin[k][:, s * 128:(s + 1) * 128],
                                         wts[(n, k)][:], start=(k == 0),
                                         stop=(k == nk - 1))
                    nc.scalar.copy(yt[s][:, n * 512:(n + 1) * 512], ps[:])
            return yt

        if upto == 0:                       # stage A only
            dump(x0t)
        if upto >= 1:
            y1 = dense(x0t, w1t, KH, "y1_", transpose_src=emb)
            if upto == 1:
                dump(y1)
        if upto >= 2:
            y1sq = mix(y1, m1, t_b1, "y1sq")
            if upto == 2:
                dump(y1sq)
        if upto >= 3:
            y2 = dense(y1sq, w2t, KI, "y2_")
            if upto == 3:
                dump(y2)
        if upto >= 4:
            y2sq = mix(y2, m2, t_b2, "y2sq")
            if upto == 4:
                dump(y2sq)
        if upto >= 5:                       # dense3 + bias + store
            for ht in range(HT):
                yo = pool.tile([128, S], f32, tag="out", bufs=2, name=f"yt{ht}")
                pss = [psum.tile([128, 512], f32, tag="mm", bufs=8,
                                 name=f"p3_{ht}_{sc}") for sc in range(2)]
                for k in range(KI):
                    wt = pool.tile([128, 128], f32r, tag="w3ring", bufs=8,
                                   name=f"w3_{ht}_{k}")
                    nc.sync.dma_start(wt[:], w3t[k * 128:(k + 1) * 128,
                                                 ht * 128:(ht + 1) * 128]
                                      .bitcast(f32r))
                    for sc in range(2):
                        nc.tensor.matmul(pss[sc][:], wt[:],
                                         y2sq[k][:, sc * 512:(sc + 1) * 512],
                                         start=(k == 0), stop=(k == KI - 1))
                for sc in range(2):
                    nc.scalar.activation(yo[:, sc * 512:(sc + 1) * 512], pss[sc][:],
                                         AF.Identity, bias=t_b3[:, ht:ht + 1])
                    nc.sync.dma_start(
                        yt_out[ht * 128:(ht + 1) * 128, sc * 512:(sc + 1) * 512],
                        yo[:, sc * 512:(sc + 1) * 512])

    nc.compile()
    return nc


_PROGRAMS = {}
LAST_RESULT = None


def _get_program(stage="full"):
    if stage not in _PROGRAMS:
        _PROGRAMS[stage] = _build_program(stage)
    return _PROGRAMS[stage]


def _prep_maps(x, word_emb, pos_emb, tok_emb, emb_ln_w, emb_ln_b,
               init_d, init_b, init_M, inter0_d, inter0_b, inter0_M,
               final_d, final_b):
    x = np.asarray(x)
    f = lambda a: np.ascontiguousarray(np.asarray(a), dtype=np.float32)
    shared = dict(
        word_emb=f(word_emb),
        posplus=f(pos_emb) + f(tok_emb)[0][None, :],
        lnw=np.ascontiguousarray(f(emb_ln_w).reshape(HT, 128).T),
        lnb=np.ascontiguousarray(f(emb_ln_b).reshape(HT, 128).T),
        w1t=np.ascontiguousarray(f(init_d).T),
        b1c=np.ascontiguousarray(f(init_b).reshape(KI, 128).T),
        m1=np.ascontiguousarray(np.asarray(init_M)).astype(ml_dtypes.bfloat16),
        w2t=np.ascontiguousarray(f(inter0_d).T),
        b2c=np.ascontiguousarray(f(inter0_b).reshape(KI, 128).T),
        m2=np.ascontiguousarray(np.asarray(inter0_M)).astype(ml_dtypes.bfloat16),
        w3t=np.ascontiguousarray(f(final_d).T),
        b3c=np.ascontiguousarray(f(final_b).reshape(HT, 128).T),
    )
    in_maps = []
    for b in range(B):
        xwb = np.ascontiguousarray(x[b].astype(np.int32).reshape(SC, 128).T)
        in_maps.append(dict(shared, xw=xwb))
    return in_maps


def kernel(**inputs):
    global LAST_RESULT
    stage = os.environ.get("KSTAGE", "full")
    ncores = int(os.environ.get("KCORES", str(N_CORES)))
    in_maps = _prep_maps(**inputs)[:ncores]
    nc = _get_program(stage)
    res = run_bass_kernel_spmd(nc, in_maps, list(range(ncores)))
    LAST_RESULT = res
    out = np.stack([res.results[b]["yt"].T for b in range(ncores)])
    if ncores < B:
        out = np.concatenate([out] + [out[:1]] * (B - ncores))
    return out
